# revision 1
# baseline (speedup 1.0000x reference)
"""Trainium2 Bass kernel for nn_DiffeqSolver: RK4 integration of
f(y) = tanh(y @ W1 + b1) @ W2 + b2 over a fixed time grid.

Sharding: data-parallel over the N=100000 points across 8 cores
(12500 points/core).  MLP weights and the time grid are replicated.

Per-core structure: points are padded to 128-point tiles and split into
two interleaved "halves" (even/odd tiles).  Each half keeps its state y
TRANSPOSED, [64 features x W points], at SBUF partitions 0:64 (point p
-> tile tt=p//128, row r=p%128; half hh=tt%2, column (tt//2)*128+r).
Per RK4 stage, per column-block (~482 cols), per half:
  - mm1: z.T[256, bn] = W1.T @ y.T via 2 f32r matmuls (K=64, M=128)
    into a 2-bank PSUM group
  - tanh on the scalar engine over the whole group in ONE op (bias b1
    fused; one op only when b1 == 0, else per-half ops)
  - mm2: k.T[64, bn] = W2.T @ h.T via 2 accumulating matmuls (K=128),
    written into the PSUM bank tanh just vacated
  - RK4 combines as fused scalar_tensor_tensor ops, split DVE/GPSIMD:
    y_new = (ys1 + 2 ys2 + ys3 - y)/3 + dt/6 k4
Matmuls run in float32r (4-byte fp32 data, 1 col/cycle PE mode; even
moving dim required, >=256 for full rate).  The whole step is emitted
stage-major over block groups so ACT/PE/DVE pipeline across blocks; the
tanh (ACT) engine is the roofline at ~93% modeled occupancy.
"""

import numpy as np

import concourse.bass as bass
import concourse.masks as masks
import concourse.mybir as mybir
import concourse.tile as tile
from concourse.bass_utils import run_bass_kernel_spmd

F32 = mybir.dt.float32
F32R = mybir.dt.float32r

N_FULL, D, H, T_FULL = 100000, 64, 256, 20
NCORES = 8

_LDW_OPT_PATCHED = False


def _enable_ldw_opt():
    """Let walrus dedupe back-to-back identical weight loads; matmuls are
    emitted weight-paired so this halves f32r self-load overhead."""
    global _LDW_OPT_PATCHED
    if _LDW_OPT_PATCHED:
        return
    import concourse.bass_utils as _bu
    _orig = _bu.run_command

    def _patched(argv, **kw):
        argv = ["--enable-ldw-opt=true" if a == "--enable-ldw-opt=false"
                else a for a in argv]
        return _orig(argv, **kw)

    _bu.run_command = _patched
    _LDW_OPT_PATCHED = True


def build_bass(npts, dts, mm_dtype=F32R, bw=512, b1_zero=False, b2_zero=False,
               ngrp=5):
    """Build the per-core Bass program.

    npts: points per core (will be padded to a multiple of 256)
    dts:  python floats, the T-1 time deltas
    """
    nsteps = len(dts)
    ntiles = -(-npts // 128)          # 128-point tiles
    if ntiles % 2:
        ntiles += 1                   # need an even tile count to pack halves
    npad = ntiles * 128
    w = npad // 2                     # packed width (columns per half)
    # Equal-size blocks, all >= 256 so f32r matmuls stream at full rate
    # (the PE runs f32r at 1/4 rate when the moving dim is < 256).
    # (also: f32r requires an even moving dim, so keep block sizes even)
    nblk = -(-w // bw)
    base = (w // nblk) // 2 * 2
    rem = w - base * nblk
    assert rem % 2 == 0
    blocks = []
    o = 0
    for i in range(nblk):
        bn = base + (2 if i < rem // 2 else 0)
        blocks.append((o, bn))
        o += bn
    assert o == w and all(bn >= 256 or nblk == 1 for _, bn in blocks), blocks

    nc = bass.Bass()
    fp = nc.dram_tensor("first_point", [npts, D], F32, kind="ExternalInput")
    w1d = nc.dram_tensor("W1", [D, H], mm_dtype, kind="ExternalInput")
    b1d = nc.dram_tensor("b1", [H], F32, kind="ExternalInput")
    w2d = nc.dram_tensor("W2", [H, D], mm_dtype, kind="ExternalInput")
    b2d = nc.dram_tensor("b2", [D], F32, kind="ExternalInput")
    outd = nc.dram_tensor("traj", [nsteps, 128, w], mm_dtype,
                          kind="ExternalOutput")

    MUL = mybir.AluOpType.mult
    ADD = mybir.AluOpType.add
    TANH = mybir.ActivationFunctionType.Tanh

    with tile.TileContext(nc) as tc:
        with (
            tc.tile_pool(name="const", bufs=1) as cpool,
            tc.tile_pool(name="state", bufs=1) as spool,
            tc.tile_pool(name="ys", bufs=5) as ypool,
            tc.tile_pool(name="hb", bufs=6) as hpool,
            tc.tile_pool(name="ld", bufs=4) as ldpool,
            tc.tile_pool(name="pz", bufs=4, space="PSUM") as pz,
        ):
            # ---- constants ----
            w1_sb = cpool.tile([64, H], mm_dtype)
            nc.sync.dma_start(w1_sb[:], w1d[:])
            w2_sb = cpool.tile([128, 128], mm_dtype)
            # W2[c*128+k, d] -> w2_sb[k, c*64+d]
            nc.sync.dma_start(w2_sb[:].rearrange("k (c d) -> k c d", c=2),
                              w2d[:].rearrange("(c k) d -> k c d", c=2))
            b1_sb = cpool.tile([128, 2], F32)
            nc.sync.dma_start(b1_sb[:], b1d[:].rearrange("(j p) -> p j", p=128))
            b2_sb = cpool.tile([64, 1], F32)
            nc.sync.dma_start(b2_sb[:], b2d[:].unsqueeze(1))
            ident = cpool.tile([128, 128], F32)
            masks.make_identity(nc, ident[:])

            # ---- state buffers ----
            # Two independent half-pipelines (even/odd 128-point tiles), both
            # living at partitions 0:64 -- matmul PSUM outputs can then always
            # target base partition 0 (dst partition offsets are rejected by
            # the compiler's ISA checks).
            ys_cur = [spool.tile([64, w], mm_dtype, name=f"ycur{h}")
                      for h in range(2)]
            ys_nxt = [spool.tile([64, w], mm_dtype, name=f"ynxt{h}")
                      for h in range(2)]
            b2s_h = spool.tile([64, 1], F32)
            b2s_1 = spool.tile([64, 1], F32)
            b2s_6 = spool.tile([64, 1], F32)

            # ---- load first_point, transpose into per-half layout ----
            nfull = npts // 128                  # full 128-row tiles
            for t in range(ntiles):
                lt = ldpool.tile([128, D], F32, tag="lt", name=f"lt{t}")
                if t < nfull:
                    nc.sync.dma_start(lt[:], fp[t * 128:(t + 1) * 128, :])
                else:
                    nc.vector.memset(lt[:], 0.0)
                    if t * 128 < npts:
                        nc.sync.dma_start(lt[0:npts - t * 128, :],
                                          fp[t * 128:npts, :])
                pt = pz.tile([64, 128], F32, tag="z", name=f"pt{t}")
                nc.tensor.transpose(pt[:], lt[:], ident[:])
                hh, b = t % 2, t // 2
                nc.vector.tensor_copy(
                    ys_cur[hh][:, b * 128:(b + 1) * 128], pt[:])

            # Block-groups swept stage-major so ACT/PE pipeline across blocks;
            # groups hide each other's stage-boundary bubbles.
            ngrp = min(ngrp, len(blocks))
            groups = [list(range(len(blocks)))[g::ngrp] for g in range(ngrp)]

            def emit_pair(st, s, j, dt, bss):
                """One RK4 stage for both 64-partition point-halves of block
                j, with matmuls interleaved so consecutive PE instructions
                share stationary weights (enables LDW dedup)."""
                bo, bn = blocks[j]
                sl = slice(bo, bo + bn)
                if s == 0:
                    for hh in range(2):
                        bs, ycur = bss[hh], ys_cur[hh]
                        if b2_zero:
                            bs["bh"] = bs["b1"] = ycur[:, sl]
                        else:
                            bh = ypool.tile([64, bw], F32, tag=f"bh{hh}",
                                            bufs=4, name=f"bh{st}_{j}_{hh}")
                            b1t = ypool.tile([64, bw], F32, tag=f"b1t{hh}",
                                             bufs=4, name=f"b1t{st}_{j}_{hh}")
                            nc.gpsimd.tensor_scalar_add(
                                bh[:, 0:bn], ycur[:, sl], b2s_h[:, 0:1])
                            nc.gpsimd.tensor_scalar_add(
                                b1t[:, 0:bn], ycur[:, sl], b2s_1[:, 0:1])
                            bs["bh"], bs["b1"] = bh[:, 0:bn], b1t[:, 0:bn]
                        bs["src"] = ycur[:, sl]
                        bs["ys"] = []

                zgs, hgs = [], []
                for hh in range(2):
                    zgs.append(pz.tile([128, 2, 512], F32, tag="z",
                                       name=f"z{st}_{j}_{s}_{hh}"))
                    hgs.append(hpool.tile([128, 2, bw], mm_dtype, tag="h",
                                          name=f"h{st}_{j}_{s}_{hh}"))
                for mh in range(2):
                    for hh in range(2):
                        nc.tensor.matmul(
                            zgs[hh][:, mh, 0:bn],
                            w1_sb[:, mh * 128:(mh + 1) * 128],
                            bss[hh]["src"], start=True, stop=True)
                for hh in range(2):
                    if b1_zero:
                        nc.scalar.activation(
                            hgs[hh][:, :, 0:bn], zgs[hh][:, :, 0:bn],
                            TANH, bias=0.0, scale=1.0)
                    else:
                        for mh in range(2):
                            nc.scalar.activation(
                                hgs[hh][:, mh, 0:bn], zgs[hh][:, mh, 0:bn],
                                TANH, bias=b1_sb[:, mh:mh + 1], scale=1.0)
                # k = h @ W2 into partitions 0:64 of each zg's bank 0, which
                # the tanh has just finished reading (saves PSUM banks)
                kts = [zgs[hh][0:64, 0, :] for hh in range(2)]
                for c in range(2):
                    for hh in range(2):
                        nc.tensor.matmul(
                            kts[hh][:, 0:bn],
                            w2_sb[:, c * 64:(c + 1) * 64],
                            hgs[hh][:, c, 0:bn],
                            start=(c == 0), stop=(c == 1),
                            skip_group_check=True)
                for hh in range(2):
                    bs, kt = bss[hh], kts[hh]
                    ycur, ynxt = ys_cur[hh], ys_nxt[hh]
                    if s < 3:
                        # ystage gates the next stage's matmuls -- on DVE
                        yst = ypool.tile([64, bw], mm_dtype, tag=f"ys{hh}",
                                         bufs=7, name=f"ys{st}_{j}_{s}_{hh}")
                        cs = dt / 2.0 if s < 2 else dt
                        nc.vector.scalar_tensor_tensor(
                            yst[:, 0:bn], kt[:, 0:bn], cs,
                            bs["bh"] if s < 2 else bs["b1"], MUL, ADD)
                        bs["src"] = yst[:, 0:bn]
                        bs["ys"].append(yst)
                        # y_new prework, split DVE/GPSIMD, off the critical
                        # path: y_new = (ys1+2ys2+ys3-y)/3 + dt/6 k4 (+b2 tm)
                        ys = bs["ys"]
                        if s == 1:
                            pacc = ypool.tile([64, bw], F32, tag=f"pa{hh}",
                                              bufs=6, name=f"pa{st}_{j}_{hh}")
                            nc.vector.scalar_tensor_tensor(
                                pacc[:, 0:bn], ys[1][:, 0:bn], 2.0,
                                ys[0][:, 0:bn], MUL, ADD)
                            bs["pa"] = pacc
                        if s == 2:
                            pacc = bs["pa"]
                            nc.gpsimd.tensor_tensor(
                                pacc[:, 0:bn], pacc[:, 0:bn], ys[2][:, 0:bn],
                                ADD)
                            nc.gpsimd.tensor_tensor(
                                pacc[:, 0:bn], pacc[:, 0:bn], ycur[:, sl],
                                mybir.AluOpType.subtract)
                    else:
                        pacc = bs["pa"]
                        nc.vector.scalar_tensor_tensor(
                            pacc[:, 0:bn], kt[:, 0:bn], dt / 2.0,
                            pacc[:, 0:bn], MUL, ADD)
                        nc.gpsimd.tensor_scalar(
                            ynxt[:, sl], pacc[:, 0:bn], 1.0 / 3.0,
                            0.0 if b2_zero else b2s_6[:, 0:1], MUL, ADD)
                        nc.sync.dma_start(
                            outd[st, hh * 64:(hh + 1) * 64, sl],
                            ynxt[:, sl])

            for st in range(nsteps):
                dt = float(dts[st])
                if not b2_zero:
                    nc.vector.tensor_scalar_mul(b2s_h[:], b2_sb[:], dt / 2.0)
                    nc.vector.tensor_scalar_mul(b2s_1[:], b2_sb[:], dt)
                    nc.vector.tensor_scalar_mul(b2s_6[:], b2_sb[:], dt / 6.0)
                for grp in groups:
                    bstate = {j: [{}, {}] for j in grp}
                    for s in range(4):
                        for j in grp:
                            emit_pair(st, s, j, dt, bstate[j])
                ys_cur, ys_nxt = ys_nxt, ys_cur
    _split_matmul_waits(nc)
    nc.finalize()
    return nc


def _split_matmul_waits(nc):
    """Self-loading (fp32/f32r) matmuls lower to an LW+MM pair whose LW
    struct can carry only one sync-wait command.  Move excess waits onto
    PE no-ops inserted right before the matmul.  Each no-op increments a
    dedicated dummy semaphore (never waited on) so CoreSim's race
    detector sees a real update."""
    # pick a semaphore id beyond everything Tile allocated
    max_id = 0
    for f in nc.m.functions:
        for blk in f.blocks:
            for inst in blk.instructions:
                si = inst.sync_info
                if si is None:
                    continue
                for wt in si.on_wait:
                    if isinstance(wt.id, int):
                        max_id = max(max_id, wt.id)
                for up in si.on_update:
                    if isinstance(up.id, int):
                        max_id = max(max_id, up.id)
    sem_id = max_id + 1
    for f in nc.m.functions:
        for blk in f.blocks:
            out = []
            n_split = 0
            for inst in blk.instructions:
                si = inst.sync_info
                if (inst.opcode != "NoOp"
                        and si is not None and len(si.on_wait) > 1):
                    waits = list(si.on_wait)
                    for wi, wt in enumerate(waits[:-1]):
                        nop = mybir.InstNoOp(
                            name=f"{inst.name}-wj{wi}", ins=[], outs=[])
                        nop.engine = inst.engine
                        nop.sync_info = mybir.SyncInfo(
                            on_wait=[wt],
                            on_update=[mybir.SyncUpdate(
                                sync_type='semaphore', id=sem_id,
                                ant_name='wj_dummy_sem',
                                update_mode='sem-inc',
                                update_value=1, update_reg=None)])
                        out.append(nop)
                    inst.sync_info = mybir.SyncInfo(
                        on_wait=[waits[-1]], on_update=list(si.on_update))
                    n_split += 1
                out.append(inst)
            if n_split:
                blk.instructions = out


def _unshard(traj, npts, nsteps):
    """[nsteps, 128, w] packed -> [nsteps, npts, D]."""
    w = traj.shape[2]
    nb = w // 128
    v = traj.reshape(nsteps, 2, 64, nb, 128)
    v = np.ascontiguousarray(v.transpose(0, 3, 1, 4, 2))
    return v.reshape(nsteps, nb * 256, 64)[:, :npts, :]


def kernel(first_point, time_steps, W1, b1, W2, b2):
    first_point = np.ascontiguousarray(first_point, dtype=np.float32)
    time_steps = np.asarray(time_steps, dtype=np.float32)
    W1 = np.ascontiguousarray(W1, dtype=np.float32)
    b1 = np.ascontiguousarray(b1, dtype=np.float32)
    W2 = np.ascontiguousarray(W2, dtype=np.float32)
    b2 = np.ascontiguousarray(b2, dtype=np.float32)

    npts = first_point.shape[0] // NCORES
    dts = [float(x) for x in np.diff(time_steps)]
    nsteps = len(dts)

    nc = build_bass(npts, dts,
                    b1_zero=not b1.any(), b2_zero=not b2.any())

    in_maps = []
    for c in range(NCORES):
        in_maps.append({
            "first_point": first_point[c * npts:(c + 1) * npts],
            "W1": W1, "b1": b1, "W2": W2, "b2": b2,
        })
    res = run_bass_kernel_spmd(nc, in_maps, core_ids=list(range(NCORES)))

    out = np.empty((nsteps + 1, first_point.shape[0], D), dtype=np.float32)
    out[0] = first_point
    for c in range(NCORES):
        out[1:, c * npts:(c + 1) * npts, :] = _unshard(
            res.results[c]["traj"], npts, nsteps)
    return out

